# revision 20
# baseline (speedup 1.0000x reference)
"""Fused attention-block kernel for Trainium2, 8-core data-parallel over batch.

Computation (see harness reference): three BN+ReLU linear branches from the
same input, attention (QK^T/16 -> softmax -> AV), then a fourth BN+ReLU
linear.  BatchNorm1d is training-mode per-channel over (batch, feature) with
channel = sequence position, so batch-sharding needs a cross-core stats
all-reduce (sync-BN); weights are replicated.

v2 design notes (vs the v1 baseline at 468us):
 - A tiny dummy AllReduce is issued right after the input-cast DMAs: the
   *first* collective pays a ~90us firmware warmup/skew cost; later ones
   take ~9-13us.  The warmup overlaps the input DMA + z-phase.
 - One merged stats AllReduce for layers 1-3 (instead of two serialized).
 - x is cast fp32->bf16 straight into SBUF (gpsimd SWDGE cast DMA, no DRAM
   round trip) and transposed by the PE during the otherwise-idle prologue
   (also warms the HAM clock gate).
 - All linear-layer biases are pre-added via K=1 rank-1 matmuls (ones row x
   bias row), which makes every BN+ReLU apply a single ACT op with
   per-partition scale/bias, and makes all BN statistics exact/direct.
 - x1/x2/r transposes go through SBUF->SBUF DMA xbar transposes (128x128
   blocks) instead of PE matmuls: frees the tensor engine + PSUM banks so
   the attention pipeline can run ~2 batches deep (pss/psr/psy pools).
 - All attention intermediates are batch-local rotating staging tiles.
 - z4 is written into x3a's storage (x3 of batch b is dead once AV(b) ran).
 - Per-batch work is spread across ACT/DVE/GPSIMD; gpsimd only runs while
   no collective is in flight (device-hang hazard).
 - Output is stored as bf16 (halves store traffic; host casts to fp32).

Hardcoded: B=256, N=256, D=256, 8 cores -> 32 batches (8192 tokens) per core.
"""
import sys
import types

sys.path.insert(0, "/opt/trn_rl_repo")

import numpy as np
import ml_dtypes
from contextlib import ExitStack

import concourse.bass as bass
import concourse.mybir as mybir
import concourse.tile as tile
from concourse.masks import make_identity

BF16 = mybir.dt.bfloat16
F32 = mybir.dt.float32
NCORES = 8
B_LOC = 32          # batches per core
T = B_LOC * 256     # tokens per core
EPS = 1e-5
NG = 2              # stats batch-group size (bn_stats free max = 512)
NCH = B_LOC // 4    # cast/transpose chunks (4 batches each)


def _install_profile_shim():
    """run_bass_kernel_spmd(trace=True) under axon needs antenv.axon_hooks,
    which this image lacks; synthesize it (harmless if tracing unused)."""
    if "antenv.axon_hooks" in sys.modules:
        return
    try:
        import antenv
        mod = types.ModuleType("antenv.axon_hooks")
        mod._hook = None
        mod.set_axon_ntff_profile_hook = lambda h: setattr(mod, "_hook", h)
        mod.get_axon_ntff_profile_hook = lambda: mod._hook
        sys.modules["antenv.axon_hooks"] = mod
        antenv.axon_hooks = mod
        from trn_agent_boot.trn_boot import _ntff_profile_via_ctypes
        hook = _ntff_profile_via_ctypes("/opt/axon/libaxon_pjrt.so")
        if hook is not None:
            mod.set_axon_ntff_profile_hook(hook)
    except Exception:
        pass


def _legalize_waits(nc, max_waits=1):
    """HW instructions carry one sync-wait slot; walrus rejects instructions
    with too many waits.  Hoist extras onto engine-matched NoOps."""
    for f in nc.m.functions:
        for bb in f.blocks:
            insts = bb.instructions
            new_list = []
            for inst in insts:
                si = inst.sync_info
                if si is not None and len(si.on_wait) > max_waits:
                    waits = list(si.on_wait)
                    extra, keep = waits[:-max_waits], waits[-max_waits:]
                    for j, w in enumerate(extra):
                        nop = mybir.InstNoOp(
                            name=f"{inst.name}-waitnop{j}",
                            engine=inst.engine,
                            ins=[], outs=[],
                            sync_info=mybir.SyncInfo(on_wait=[w], on_update=[]),
                        )
                        nc.register_instruction(nop, overwrite=True)
                        new_list.append(nop)
                    inst.sync_info = mybir.SyncInfo(
                        on_wait=keep, on_update=list(si.on_update))
                new_list.append(inst)
            del insts[:]
            for x in new_list:
                insts.append(x)


def build_program(use_collectives=True):
    nc = bass.Bass("TRN2", target_bir_lowering=False, debug=False,
                   num_devices=NCORES)

    def act_copy(out, in_):
        nc.scalar.activation(out=out, in_=in_,
                             func=mybir.ActivationFunctionType.Copy)

    x_d = nc.dram_tensor("x", [T, 256], F32, kind="ExternalInput")
    w123_d = nc.dram_tensor("w123", [128, 2, 768], BF16, kind="ExternalInput")
    w4_d = nc.dram_tensor("w4", [128, 2, 256], BF16, kind="ExternalInput")
    brows_d = nc.dram_tensor("brows", [1, 1024], BF16, kind="ExternalInput")
    gb_d = nc.dram_tensor("gb", [128, 2, 2], F32, kind="ExternalInput")
    out_d = nc.dram_tensor("out", [T, 256], BF16, kind="ExternalOutput")


    groups = [list(range(NCORES))]
    # x rows are (b, h, p); chunks of 4 batches = 8 bh-blocks
    x_r = x_d.ap().rearrange("(c q p) e -> p c q e", c=NCH, q=8, p=128)
    out_r = out_d.ap().rearrange("(b h p) e -> p b h e", b=B_LOC, h=2, p=128)

    with ExitStack() as ctx:
        tc = ctx.enter_context(tile.TileContext(nc))
        big = ctx.enter_context(tc.tile_pool(name="big", bufs=1))
        small = ctx.enter_context(tc.tile_pool(name="small", bufs=1))
        stage = ctx.enter_context(tc.tile_pool(name="stage", bufs=3))
        dram = ctx.enter_context(tc.tile_pool(name="dram", bufs=1, space="DRAM"))

        # ---- persistent big tiles --------------------------------------
        xT = big.tile([128, 2, T], BF16, tag="xT")              # x d-major
        z12b = big.tile([128, B_LOC, 2, 512], BF16, tag="z12")  # l1/l2 interleaved
        z3b = big.tile([128, B_LOC, 2, 256], BF16, tag="z3")
        x3a = big.tile([128, B_LOC, 2, 257], BF16, tag="x3a")   # +ones col 256

        # ---- constants --------------------------------------------------
        w123 = small.tile([128, 2, 768], BF16, tag="w123")
        w4 = small.tile([128, 2, 256], BF16, tag="w4")
        brows = small.tile([1, 1024], BF16, tag="brows")
        gbt = small.tile([128, 2, 2], F32, tag="gbt")
        idn = small.tile([128, 128], BF16, tag="idn")
        onesr = small.tile([1, 128], BF16, tag="onesr")
        nc.sync.dma_start(out=w123[:], in_=w123_d.ap())
        nc.sync.dma_start(out=w4[:], in_=w4_d.ap())
        nc.sync.dma_start(out=brows[:], in_=brows_d.ap())
        nc.sync.dma_start(out=gbt[:], in_=gb_d.ap())
        make_identity(nc, idn[:])
        nc.vector.memset(onesr[:], 1.0)
        nc.vector.memset(x3a[:, :, :, 256:257], 1.0)
        b12row = brows[:, 0:512]      # b1/b2 interleaved
        b3row = brows[:, 512:768]
        b4row = brows[:, 768:1024]

        # ---- x cast DMAs (gpsimd SWDGE), then dummy warmup AllReduce ----
        # Casts are emitted strictly before the collective trigger: pool-
        # engine work concurrent with an in-flight collective hangs.
        xstg = []
        for c in range(NCH):
            xs = stage.tile([128, 8, 256], BF16, tag="xstg", bufs=2,
                            name=f"xs{c}")
            nc.gpsimd.dma_start(out=xs[:], in_=x_r[:, c, :, :])
            xstg.append(xs)
        ar_w_i = dram.tile([128, 2], F32, tag="arwi")
        ar_w_o = dram.tile([128, 2], F32, tag="arwo")
        if use_collectives:
            nc.gpsimd.collective_compute(
                "AllReduce", mybir.AluOpType.add, replica_groups=groups,
                ins=[ar_w_i[:].opt()], outs=[ar_w_o[:].opt()])

        # ---- P0+P1 interleaved per chunk: transpose x, z12+z3 matmuls ---
        st12 = small.tile([128, B_LOC, 2, 6], F32, tag="st12")  # per-batch rows
        st3 = small.tile([128, B_LOC, 2, 6], F32, tag="st3")
        trp_cm = tc.tile_pool(name="trp", bufs=2, space="PSUM")
        trp = trp_cm.__enter__()
        ps12_cm = tc.tile_pool(name="ps12", bufs=2, space="PSUM")
        ps12 = ps12_cm.__enter__()
        ps3_cm = tc.tile_pool(name="ps3", bufs=2, space="PSUM")
        ps3 = ps3_cm.__enter__()
        for c in range(NCH):
            for q in range(8):
                bh = c * 8 + q
                pt0 = trp.tile([128, 2, 128], BF16, tag="trp", name=f"tp{bh}")
                for dc in range(2):
                    nc.tensor.transpose(
                        out=pt0[:, dc, :],
                        in_=xstg[c][:, q, dc * 128:(dc + 1) * 128],
                        identity=idn[:])
                if bh % 2 == 0:
                    nc.vector.tensor_copy(out=xT[:, :, bh * 128:(bh + 1) * 128],
                                          in_=pt0[:])
                else:
                    act_copy(xT[:, :, bh * 128:(bh + 1) * 128], pt0[:])
            for b in range(c * 4, (c + 1) * 4):
                psz = ps12.tile([128, 2, 512], F32, tag="ps12", name=f"pz{b}")
                for h in range(2):
                    nc.tensor.matmul(out=psz[:, h, :], lhsT=onesr[:],
                                     rhs=b12row, start=True, stop=False)
                    for dc in range(2):
                        nc.tensor.matmul(
                            out=psz[:, h, :],
                            lhsT=xT[:, dc, b * 256 + h * 128: b * 256 + (h + 1) * 128],
                            rhs=w123[:, dc, 0:512],
                            start=False, stop=(dc == 1))
                act_copy(z12b[:, b, :, :], psz[:])

                psz3 = ps3.tile([128, 2, 256], F32, tag="ps3", name=f"pz3{b}")
                for h in range(2):
                    nc.tensor.matmul(out=psz3[:, h, :], lhsT=onesr[:],
                                     rhs=b3row, start=True, stop=False)
                    for dc in range(2):
                        nc.tensor.matmul(
                            out=psz3[:, h, :],
                            lhsT=xT[:, dc, b * 256 + h * 128: b * 256 + (h + 1) * 128],
                            rhs=w123[:, dc, 512:768],
                            start=False, stop=(dc == 1))
                act_copy(z3b[:, b, :, :], psz3[:])
                for h in range(2):
                    nc.vector.bn_stats(out=st12[:, b, h, :],
                                       in_=z12b[:, b, h, :])
                    nc.vector.bn_stats(out=st3[:, b, h, :],
                                       in_=z3b[:, b, h, :])
        ps3_cm.__exit__(None, None, None)
        ps12_cm.__exit__(None, None, None)
        trp_cm.__exit__(None, None, None)

        # ---- stats aggregation: per channel mean & E[z^2] ---------------
        def stats_cols_il(lidx, st, arin, col0):
            """st: [128, B_LOC, 2, 6] bn_stats rows of interleaved 512-wide
            inputs; stream lidx(0/1) = cols 3l+1 (mean), 3l+2 (count*var),
            count=256.  arin[:, col0+2h]=mean, +2h+1=E[z^2] (core-local)."""
            for h in range(2):
                msum = small.tile([128, 1], F32, tag=f"ms{lidx}_{h}",
                                  name=f"ms{lidx}{h}")
                nc.vector.tensor_reduce(
                    out=msum[:], in_=st[:, :, h, 3 * lidx + 1:3 * lidx + 2],
                    axis=mybir.AxisListType.XY, op=mybir.AluOpType.add)
                sq = small.tile([128, B_LOC, 1], F32, tag=f"sq{lidx}_{h}",
                                name=f"sq{lidx}{h}")
                nc.vector.tensor_tensor(
                    out=sq[:], in0=st[:, :, h, 3 * lidx + 1:3 * lidx + 2],
                    in1=st[:, :, h, 3 * lidx + 1:3 * lidx + 2],
                    op=mybir.AluOpType.mult)
                sqsum = small.tile([128, 1], F32, tag=f"qs{lidx}_{h}",
                                   name=f"qs{lidx}{h}")
                nc.vector.tensor_reduce(out=sqsum[:], in_=sq[:],
                                        axis=mybir.AxisListType.XY,
                                        op=mybir.AluOpType.add)
                cvsum = small.tile([128, 1], F32, tag=f"cv{lidx}_{h}",
                                   name=f"cv{lidx}{h}")
                nc.vector.tensor_reduce(
                    out=cvsum[:], in_=st[:, :, h, 3 * lidx + 2:3 * lidx + 3],
                    axis=mybir.AxisListType.XY, op=mybir.AluOpType.add)
                nc.vector.tensor_scalar_mul(
                    arin[:, col0 + 2 * h:col0 + 2 * h + 1], msum[:], 1.0 / B_LOC)
                nc.vector.tensor_scalar_mul(cvsum[:], cvsum[:],
                                            1.0 / (256 * B_LOC))
                nc.vector.tensor_scalar_mul(
                    arin[:, col0 + 2 * h + 1:col0 + 2 * h + 2], sqsum[:],
                    1.0 / B_LOC)
                nc.vector.tensor_tensor(
                    out=arin[:, col0 + 2 * h + 1:col0 + 2 * h + 2],
                    in0=arin[:, col0 + 2 * h + 1:col0 + 2 * h + 2],
                    in1=cvsum[:], op=mybir.AluOpType.add)

        def stats_cols(lidx, st, arin, col0, nfree):
            """st: [128, B_LOC, 2, 6] per-batch bn_stats rows over 256-wide
            inputs (even/odd streams, count = nfree/2 each).  Writes per h:
            arin[:, col0+2h] = core mean, +2h+1 = core E[z^2]."""
            ngrp = 2 * B_LOC  # even/odd x batches
            for h in range(2):
                msum = small.tile([128, 1], F32, tag=f"Ms{lidx}_{h}",
                                  name=f"Ms{lidx}{h}")
                nc.vector.tensor_reduce(out=msum[:], in_=st[:, :, h, 1:5:3],
                                        axis=mybir.AxisListType.XY,
                                        op=mybir.AluOpType.add)
                sq = small.tile([128, B_LOC, 2], F32, tag=f"Sq{lidx}_{h}",
                                name=f"Sq{lidx}{h}")
                nc.vector.tensor_tensor(out=sq[:], in0=st[:, :, h, 1:5:3],
                                        in1=st[:, :, h, 1:5:3],
                                        op=mybir.AluOpType.mult)
                sqsum = small.tile([128, 1], F32, tag=f"Qs{lidx}_{h}",
                                   name=f"Qs{lidx}{h}")
                nc.vector.tensor_reduce(out=sqsum[:], in_=sq[:],
                                        axis=mybir.AxisListType.XY,
                                        op=mybir.AluOpType.add)
                cvsum = small.tile([128, 1], F32, tag=f"Cv{lidx}_{h}",
                                   name=f"Cv{lidx}{h}")
                nc.vector.tensor_reduce(out=cvsum[:], in_=st[:, :, h, 2:6:3],
                                        axis=mybir.AxisListType.XY,
                                        op=mybir.AluOpType.add)
                nc.vector.tensor_scalar_mul(
                    arin[:, col0 + 2 * h:col0 + 2 * h + 1], msum[:], 1.0 / ngrp)
                nc.vector.tensor_scalar_mul(
                    cvsum[:], cvsum[:], 1.0 / (ngrp * nfree // 2))
                nc.vector.tensor_scalar_mul(
                    arin[:, col0 + 2 * h + 1:col0 + 2 * h + 2], sqsum[:],
                    1.0 / ngrp)
                nc.vector.tensor_tensor(
                    out=arin[:, col0 + 2 * h + 1:col0 + 2 * h + 2],
                    in0=arin[:, col0 + 2 * h + 1:col0 + 2 * h + 2],
                    in1=cvsum[:], op=mybir.AluOpType.add)

        def emit_allreduce(lidx, arin, width):
            ar_i = dram.tile([128, width], F32, tag=f"ari{lidx}", name=f"ai{lidx}")
            ar_o = dram.tile([128, width], F32, tag=f"aro{lidx}", name=f"ao{lidx}")
            nc.sync.dma_start(out=ar_i[:], in_=arin[:])
            if use_collectives:
                nc.gpsimd.collective_compute(
                    "AllReduce", mybir.AluOpType.add, replica_groups=groups,
                    ins=[ar_i[:].opt()], outs=[ar_o[:].opt()])
            else:
                nc.gpsimd.dma_start(out=ar_o[:], in_=ar_i[:])
            artot = small.tile([128, width], F32, tag=f"artot{lidx}",
                               name=f"at{lidx}")
            nc.sync.dma_start(out=artot[:], in_=ar_o[:])
            return artot

        def bn_finalize(lidx, artot):
            """artot cols per half h: 2h = sum of core means, 2h+1 = sum of
            core E[z^2] (z includes bias).  -> s, c [128, 2] f32."""
            meanz = small.tile([128, 2], F32, tag=f"mz{lidx}", name=f"mz{lidx}")
            nc.vector.tensor_scalar_mul(meanz[:], artot[:, 0:4:2], 1.0 / NCORES)
            ey2 = small.tile([128, 2], F32, tag=f"ey{lidx}", name=f"ey{lidx}")
            nc.vector.tensor_scalar_mul(ey2[:], artot[:, 1:4:2], 1.0 / NCORES)
            varz = small.tile([128, 2], F32, tag=f"vz{lidx}", name=f"vz{lidx}")
            m2 = small.tile([128, 2], F32, tag=f"m2{lidx}", name=f"m2{lidx}")
            nc.vector.tensor_tensor(out=m2[:], in0=meanz[:], in1=meanz[:],
                                    op=mybir.AluOpType.mult)
            nc.vector.tensor_tensor(out=varz[:], in0=ey2[:], in1=m2[:],
                                    op=mybir.AluOpType.subtract)
            nc.vector.tensor_scalar_add(varz[:], varz[:], EPS)
            sd = small.tile([128, 2], F32, tag=f"sd{lidx}", name=f"sd{lidx}")
            nc.scalar.sqrt(out=sd[:], in_=varz[:])
            rstd = small.tile([128, 2], F32, tag=f"rs{lidx}", name=f"rs{lidx}")
            nc.vector.reciprocal(out=rstd[:], in_=sd[:])
            s = small.tile([128, 2], F32, tag=f"s{lidx}", name=f"s{lidx}")
            nc.vector.tensor_tensor(out=s[:], in0=rstd[:], in1=gbt[:, :, 0],
                                    op=mybir.AluOpType.mult)
            c = small.tile([128, 2], F32, tag=f"c{lidx}", name=f"c{lidx}")
            nc.vector.tensor_tensor(out=c[:], in0=meanz[:], in1=s[:],
                                    op=mybir.AluOpType.mult)
            nc.vector.tensor_tensor(out=c[:], in0=gbt[:, :, 1], in1=c[:],
                                    op=mybir.AluOpType.subtract)
            return s, c

        arin123 = small.tile([128, 12], F32, tag="arin123")
        stats_cols_il(0, st12, arin123, 0)
        stats_cols_il(1, st12, arin123, 4)
        stats_cols(2, st3, arin123, 8, 256)
        artot123 = emit_allreduce(123, arin123, 12)
        s_l, c_l = [None] * 4, [None] * 4
        s_l[0], c_l[0] = bn_finalize(0, artot123[:, 0:4])
        s_l[1], c_l[1] = bn_finalize(1, artot123[:, 4:8])
        s_l[2], c_l[2] = bn_finalize(2, artot123[:, 8:12])

        # ---- P3: per-batch apply + transposes + attention + layer4 ------
        st4 = small.tile([128, B_LOC, 2, 6], F32, tag="st12")  # aliases st12 (dead)

        pss_cm = tc.tile_pool(name="pss", bufs=2, space="PSUM")
        pss_p = pss_cm.__enter__()
        psr_cm = tc.tile_pool(name="psr", bufs=2, space="PSUM")
        psr_p = psr_cm.__enter__()
        psy_cm = tc.tile_pool(name="psy", bufs=2, space="PSUM")
        psy_p = psy_cm.__enter__()
        ptr_cm = tc.tile_pool(name="ptr", bufs=2, space="PSUM")
        ptr_p = ptr_cm.__enter__()

        for b in range(B_LOC):
            # -- BN+ReLU applies: x1 (ACT), x2 (GPSIMD), x3 (ACT) --------
            apl = stage.tile([128, 2, 2, 256], BF16, tag="apl", name=f"ap{b}")
            for h in range(2):
                nc.scalar.activation(
                    out=apl[:, 0, h, :], in_=z12b[:, b, h, 0:512:2],
                    func=mybir.ActivationFunctionType.Relu,
                    scale=s_l[0][:, h:h + 1], bias=c_l[0][:, h:h + 1])
            for h in range(2):
                nc.gpsimd.tensor_scalar(
                    out=apl[:, 1, h, :], in0=z12b[:, b, h, 1:512:2],
                    scalar1=s_l[1][:, h:h + 1], scalar2=c_l[1][:, h:h + 1],
                    op0=mybir.AluOpType.mult, op1=mybir.AluOpType.add)
                nc.gpsimd.tensor_scalar_max(apl[:, 1, h, :], apl[:, 1, h, :],
                                            0.0)
            for h in range(2):
                nc.scalar.activation(
                    out=x3a[:, b, h, 0:256], in_=z3b[:, b, h, :],
                    func=mybir.ActivationFunctionType.Relu,
                    scale=s_l[2][:, h:h + 1], bias=c_l[2][:, h:h + 1])
            # -- transpose x1/x2 via PE (bf16 psum) -----------------------
            x1t = stage.tile([128, 2, 256], BF16, tag="x1t", name=f"x1t{b}")
            x2t = stage.tile([128, 2, 256], BF16, tag="x2t", name=f"x2t{b}")
            for l, xlt in ((0, x1t), (1, x2t)):
                pst = ptr_p.tile([128, 2, 2, 128], BF16, tag="ptr",
                                 name=f"ptx{b}_{l}")
                for h in range(2):
                    for dc in range(2):
                        nc.tensor.transpose(
                            out=pst[:, dc, h, :],
                            in_=apl[:, l, h, dc * 128:(dc + 1) * 128],
                            identity=idn[:])
                if l == 0:
                    nc.vector.tensor_copy(
                        out=xlt[:],
                        in_=pst[:].rearrange("p dc h t -> p dc (h t)"))
                else:
                    act_copy(xlt[:],
                             pst[:].rearrange("p dc h t -> p dc (h t)"))
            # -- S^T = x2^T x1 (per batch) --------------------------------
            pss = pss_p.tile([128, 2, 256], F32, tag="pss", name=f"ss{b}")
            for mc in range(2):
                for ec in range(2):
                    nc.tensor.matmul(
                        out=pss[:, mc, :],
                        lhsT=x2t[:, ec, mc * 128:(mc + 1) * 128],
                        rhs=x1t[:, ec, :],
                        start=(ec == 0), stop=(ec == 1))
            pt = stage.tile([128, 2, 256], BF16, tag="pt", name=f"pt{b}")
            nc.scalar.activation(out=pt[:], in_=pss[:], scale=1.0 / 16.0,
                                 func=mybir.ActivationFunctionType.Exp)
            # -- AV with ones column -> row sums in col 256 ---------------
            rst = stage.tile([128, 2, 257], BF16, tag="rst", name=f"rt{b}")
            invr2 = stage.tile([128, 2], F32, tag="invr", name=f"iv{b}")
            for nc_ in range(2):
                psr = psr_p.tile([128, 257], F32, tag="psr", name=f"pr{b}_{nc_}")
                for mc in range(2):
                    nc.tensor.matmul(
                        out=psr[:, 0:257],
                        lhsT=pt[:, mc, nc_ * 128:(nc_ + 1) * 128],
                        rhs=x3a[:, b, mc, 0:257],
                        start=(mc == 0), stop=(mc == 1))
                nc.vector.reciprocal(out=invr2[:, nc_:nc_ + 1],
                                     in_=psr[:, 256:257])
                nc.vector.tensor_scalar_mul(rst[:, nc_, 0:256], psr[:, 0:256],
                                            invr2[:, nc_:nc_ + 1])
            # -- rT via PE transpose (bf16 psum) --------------------------
            rT = stage.tile([128, 2, 256], BF16, tag="rT", name=f"rr{b}")
            pstr = ptr_p.tile([128, 2, 2, 128], BF16, tag="ptr",
                              name=f"ptr{b}")
            for nc_ in range(2):
                for dc in range(2):
                    nc.tensor.transpose(
                        out=pstr[:, dc, nc_, :],
                        in_=rst[:, nc_, dc * 128:(dc + 1) * 128],
                        identity=idn[:])
            nc.vector.tensor_copy(
                out=rT[:], in_=pstr[:].rearrange("p dc n t -> p dc (n t)"))
            # -- layer 4 (+b4 rank-1), z4 -> x3a storage, stats -----------
            psy = psy_p.tile([128, 2, 256], F32, tag="psy", name=f"py{b}")
            for h in range(2):
                nc.tensor.matmul(out=psy[:, h, :], lhsT=onesr[:], rhs=b4row,
                                 start=True, stop=False)
                for dc in range(2):
                    nc.tensor.matmul(
                        out=psy[:, h, :],
                        lhsT=rT[:, dc, h * 128:(h + 1) * 128],
                        rhs=w4[:, dc, :],
                        start=False, stop=(dc == 1))
            if b % 2 == 0:
                act_copy(x3a[:, b, :, 0:256], psy[:])
            else:
                nc.vector.tensor_copy(out=x3a[:, b, :, 0:256], in_=psy[:])
            for h in range(2):
                nc.vector.bn_stats(out=st4[:, b, h, :],
                                   in_=x3a[:, b, h, 0:256])

        ptr_cm.__exit__(None, None, None)
        psy_cm.__exit__(None, None, None)
        psr_cm.__exit__(None, None, None)
        pss_cm.__exit__(None, None, None)

        # ---- final BN stats AR + apply + store --------------------------
        arin4 = small.tile([128, 4], F32, tag="arin4")
        stats_cols(4, st4, arin4, 0, 256)
        artot4 = emit_allreduce(4, arin4, 4)
        s_l[3], c_l[3] = bn_finalize(3, artot4)
        for b in range(B_LOC):
            ost = stage.tile([128, 2, 256], BF16, tag="ost", bufs=3,
                             name=f"os{b}")
            for h in range(2):
                nc.scalar.activation(
                    out=ost[:, h, :], in_=x3a[:, b, h, 0:256],
                    func=mybir.ActivationFunctionType.Relu,
                    scale=s_l[3][:, h:h + 1], bias=c_l[3][:, h:h + 1])
            eng = nc.sync if b % 2 == 0 else nc.scalar
            eng.dma_start(out=out_r[:, b, :, :], in_=ost[:])

    _legalize_waits(nc)
    return nc


_CACHE = {}


def _prep_core_inputs(inputs):
    bf = ml_dtypes.bfloat16
    W = [inputs["W1"], inputs["W2"], inputs["W3"], inputs["W4"]]
    bs = [inputs["b1"], inputs["b2"], inputs["b3"], inputs["b4"]]
    gamma, beta = inputs["gamma"], inputs["beta"]

    w123 = np.zeros((128, 2, 768), dtype=bf)
    for c in range(2):
        w123[:, c, 0:512:2] = W[0][:, c * 128:(c + 1) * 128].T.astype(bf)
        w123[:, c, 1:512:2] = W[1][:, c * 128:(c + 1) * 128].T.astype(bf)
        w123[:, c, 512:768] = W[2][:, c * 128:(c + 1) * 128].T.astype(bf)
    w4 = np.zeros((128, 2, 256), dtype=bf)
    for c in range(2):
        w4[:, c, :] = W[3][:, c * 128:(c + 1) * 128].T.astype(bf)
    brows = np.zeros((1, 1024), dtype=bf)
    brows[0, 0:512:2] = bs[0].astype(bf)
    brows[0, 1:512:2] = bs[1].astype(bf)
    brows[0, 512:768] = bs[2].astype(bf)
    brows[0, 768:1024] = bs[3].astype(bf)
    gb = np.zeros((128, 2, 2), dtype=np.float32)
    for h in range(2):
        gb[:, h, 0] = gamma[h * 128:(h + 1) * 128]
        gb[:, h, 1] = beta[h * 128:(h + 1) * 128]
    return w123, w4, brows, gb


def kernel(**inputs):
    _install_profile_shim()
    from concourse.bass_utils import run_bass_kernel_spmd

    if "nc" not in _CACHE:
        _CACHE["nc"] = build_program()
    nc = _CACHE["nc"]

    x = np.asarray(inputs["x"], dtype=np.float32)
    w123, w4, brows, gb = _prep_core_inputs(
        {k: np.asarray(v) for k, v in inputs.items()})

    in_maps = []
    for i in range(NCORES):
        xs = np.ascontiguousarray(
            x[i * B_LOC:(i + 1) * B_LOC].reshape(T, 256))
        in_maps.append({"x": xs, "w123": w123, "w4": w4, "brows": brows,
                        "gb": gb})

    trace = _CACHE.get("trace", False)
    res = run_bass_kernel_spmd(nc, in_maps, list(range(NCORES)), trace=trace)
    _CACHE["last_result"] = res

    out = np.empty((256, 256, 256), dtype=np.float32)
    for i in range(NCORES):
        out[i * B_LOC:(i + 1) * B_LOC] = np.asarray(
            res.results[i]["out"]).astype(np.float32).reshape(B_LOC, 256, 256)
    return out


# revision 21
# speedup vs baseline: 1.1199x; 1.1199x over previous
"""Fused attention-block kernel for Trainium2, 8-core data-parallel over batch.

Computation (see harness reference): three BN+ReLU linear branches from the
same input, attention (QK^T/16 -> softmax -> AV), then a fourth BN+ReLU
linear.  BatchNorm1d is training-mode per-channel over (batch, feature) with
channel = sequence position, so batch-sharding needs a cross-core stats
all-reduce (sync-BN); weights are replicated.

v2 design notes (vs the v1 baseline at 468us):
 - A tiny dummy AllReduce is issued right after the input-cast DMAs: the
   *first* collective pays a ~90us firmware warmup/skew cost; later ones
   take ~9-13us.  The warmup overlaps the input DMA + z-phase.
 - One merged stats AllReduce for layers 1-3 (instead of two serialized).
 - x is cast fp32->bf16 straight into SBUF (gpsimd SWDGE cast DMA, no DRAM
   round trip) and transposed by the PE during the otherwise-idle prologue
   (also warms the HAM clock gate).
 - All linear-layer biases are pre-added via K=1 rank-1 matmuls (ones row x
   bias row), which makes every BN+ReLU apply a single ACT op with
   per-partition scale/bias, and makes all BN statistics exact/direct.
 - x1/x2/r transposes go through SBUF->SBUF DMA xbar transposes (128x128
   blocks) instead of PE matmuls: frees the tensor engine + PSUM banks so
   the attention pipeline can run ~2 batches deep (pss/psr/psy pools).
 - All attention intermediates are batch-local rotating staging tiles.
 - z4 is written into x3a's storage (x3 of batch b is dead once AV(b) ran).
 - Per-batch work is spread across ACT/DVE/GPSIMD; gpsimd only runs while
   no collective is in flight (device-hang hazard).
 - Output is stored as bf16 (halves store traffic; host casts to fp32).

Hardcoded: B=256, N=256, D=256, 8 cores -> 32 batches (8192 tokens) per core.
"""
import sys
import types

sys.path.insert(0, "/opt/trn_rl_repo")

import numpy as np
import ml_dtypes
from contextlib import ExitStack

import concourse.bass as bass
import concourse.mybir as mybir
import concourse.tile as tile
from concourse.masks import make_identity

BF16 = mybir.dt.bfloat16
F32 = mybir.dt.float32
NCORES = 8
B_LOC = 32          # batches per core
T = B_LOC * 256     # tokens per core
EPS = 1e-5
NG = 2              # stats batch-group size (bn_stats free max = 512)
NCH = B_LOC // 4    # cast/transpose chunks (4 batches each)


def _install_profile_shim():
    """run_bass_kernel_spmd(trace=True) under axon needs antenv.axon_hooks,
    which this image lacks; synthesize it (harmless if tracing unused)."""
    if "antenv.axon_hooks" in sys.modules:
        return
    try:
        import antenv
        mod = types.ModuleType("antenv.axon_hooks")
        mod._hook = None
        mod.set_axon_ntff_profile_hook = lambda h: setattr(mod, "_hook", h)
        mod.get_axon_ntff_profile_hook = lambda: mod._hook
        sys.modules["antenv.axon_hooks"] = mod
        antenv.axon_hooks = mod
        from trn_agent_boot.trn_boot import _ntff_profile_via_ctypes
        hook = _ntff_profile_via_ctypes("/opt/axon/libaxon_pjrt.so")
        if hook is not None:
            mod.set_axon_ntff_profile_hook(hook)
    except Exception:
        pass


def _legalize_waits(nc, max_waits=1):
    """HW instructions carry one sync-wait slot; walrus rejects instructions
    with too many waits.  Hoist extras onto engine-matched NoOps."""
    for f in nc.m.functions:
        for bb in f.blocks:
            insts = bb.instructions
            new_list = []
            for inst in insts:
                si = inst.sync_info
                if si is not None and len(si.on_wait) > max_waits:
                    waits = list(si.on_wait)
                    extra, keep = waits[:-max_waits], waits[-max_waits:]
                    for j, w in enumerate(extra):
                        nop = mybir.InstNoOp(
                            name=f"{inst.name}-waitnop{j}",
                            engine=inst.engine,
                            ins=[], outs=[],
                            sync_info=mybir.SyncInfo(on_wait=[w], on_update=[]),
                        )
                        nc.register_instruction(nop, overwrite=True)
                        new_list.append(nop)
                    inst.sync_info = mybir.SyncInfo(
                        on_wait=keep, on_update=list(si.on_update))
                new_list.append(inst)
            del insts[:]
            for x in new_list:
                insts.append(x)


def build_program(use_collectives=True):
    nc = bass.Bass("TRN2", target_bir_lowering=False, debug=False,
                   num_devices=NCORES)

    def act_copy(out, in_):
        nc.scalar.activation(out=out, in_=in_,
                             func=mybir.ActivationFunctionType.Copy)

    x_d = nc.dram_tensor("x", [T, 256], F32, kind="ExternalInput")
    w123_d = nc.dram_tensor("w123", [128, 2, 768], BF16, kind="ExternalInput")
    w4_d = nc.dram_tensor("w4", [128, 2, 256], BF16, kind="ExternalInput")
    brows_d = nc.dram_tensor("brows", [1, 1024], BF16, kind="ExternalInput")
    gb_d = nc.dram_tensor("gb", [128, 2, 2], F32, kind="ExternalInput")
    out_d = nc.dram_tensor("out", [T, 256], BF16, kind="ExternalOutput")


    groups = [list(range(NCORES))]
    # x rows are (b, h, p); chunks of 4 batches = 8 bh-blocks
    x_r = x_d.ap().rearrange("(c q p) e -> p c q e", c=NCH, q=8, p=128)
    out_r = out_d.ap().rearrange("(b h p) e -> p b h e", b=B_LOC, h=2, p=128)

    with ExitStack() as ctx:
        tc = ctx.enter_context(tile.TileContext(nc))
        big = ctx.enter_context(tc.tile_pool(name="big", bufs=1))
        small = ctx.enter_context(tc.tile_pool(name="small", bufs=1))
        stage = ctx.enter_context(tc.tile_pool(name="stage", bufs=3))
        dram = ctx.enter_context(tc.tile_pool(name="dram", bufs=1, space="DRAM"))

        # ---- persistent big tiles --------------------------------------
        xT = big.tile([128, 2, T], BF16, tag="xT")              # x d-major
        z12b = big.tile([128, B_LOC, 2, 512], BF16, tag="z12")  # l1/l2 interleaved
        z3b = big.tile([128, B_LOC, 2, 256], BF16, tag="z3")
        x3a = big.tile([128, B_LOC, 2, 257], BF16, tag="x3a")   # +ones col 256

        # ---- constants --------------------------------------------------
        w123 = small.tile([128, 2, 768], BF16, tag="w123")
        w4 = small.tile([128, 2, 256], BF16, tag="w4")
        brows = small.tile([1, 1024], BF16, tag="brows")
        gbt = small.tile([128, 2, 2], F32, tag="gbt")
        idn = small.tile([128, 128], BF16, tag="idn")
        onesr = small.tile([1, 128], BF16, tag="onesr")
        nc.sync.dma_start(out=w123[:], in_=w123_d.ap())
        nc.sync.dma_start(out=w4[:], in_=w4_d.ap())
        nc.sync.dma_start(out=brows[:], in_=brows_d.ap())
        nc.sync.dma_start(out=gbt[:], in_=gb_d.ap())
        make_identity(nc, idn[:])
        nc.vector.memset(onesr[:], 1.0)
        nc.vector.memset(x3a[:, :, :, 256:257], 1.0)
        b12row = brows[:, 0:512]      # b1/b2 interleaved
        b3row = brows[:, 512:768]
        b4row = brows[:, 768:1024]

        # ---- x cast DMAs (gpsimd SWDGE), then dummy warmup AllReduce ----
        # Casts are emitted strictly before the collective trigger: pool-
        # engine work concurrent with an in-flight collective hangs.
        xstg = []
        for c in range(NCH):
            xs = stage.tile([128, 8, 256], BF16, tag="xstg", bufs=2,
                            name=f"xs{c}")
            nc.gpsimd.dma_start(out=xs[:], in_=x_r[:, c, :, :])
            xstg.append(xs)
        ar_w_i = dram.tile([128, 2], F32, tag="arwi")
        ar_w_o = dram.tile([128, 2], F32, tag="arwo")
        if use_collectives:
            nc.gpsimd.collective_compute(
                "AllReduce", mybir.AluOpType.add, replica_groups=groups,
                ins=[ar_w_i[:].opt()], outs=[ar_w_o[:].opt()])

        # ---- P0+P1 interleaved per chunk: transpose x, z12+z3 matmuls ---
        st12 = small.tile([128, B_LOC, 2, 6], F32, tag="st12")  # per-batch rows
        st3 = small.tile([128, B_LOC, 2, 6], F32, tag="st3")
        trp_cm = tc.tile_pool(name="trp", bufs=2, space="PSUM")
        trp = trp_cm.__enter__()
        ps12_cm = tc.tile_pool(name="ps12", bufs=4, space="PSUM")
        ps12 = ps12_cm.__enter__()
        ps3_cm = tc.tile_pool(name="ps3", bufs=2, space="PSUM")
        ps3 = ps3_cm.__enter__()
        for c in range(NCH):
            for q in range(8):
                bh = c * 8 + q
                pt0 = trp.tile([128, 2, 128], BF16, tag="trp", name=f"tp{bh}")
                for dc in range(2):
                    nc.tensor.transpose(
                        out=pt0[:, dc, :],
                        in_=xstg[c][:, q, dc * 128:(dc + 1) * 128],
                        identity=idn[:])
                if bh % 2 == 0:
                    nc.vector.tensor_copy(out=xT[:, :, bh * 128:(bh + 1) * 128],
                                          in_=pt0[:])
                else:
                    act_copy(xT[:, :, bh * 128:(bh + 1) * 128], pt0[:])
            for b in range(c * 4, (c + 1) * 4):
                for h in range(2):
                    psz = ps12.tile([128, 512], F32, tag="ps12",
                                    name=f"pz{b}_{h}")
                    nc.tensor.matmul(out=psz[:], lhsT=onesr[:],
                                     rhs=b12row, start=True, stop=False)
                    for dc in range(2):
                        nc.tensor.matmul(
                            out=psz[:],
                            lhsT=xT[:, dc, b * 256 + h * 128: b * 256 + (h + 1) * 128],
                            rhs=w123[:, dc, 0:512],
                            start=False, stop=(dc == 1))
                    act_copy(z12b[:, b, h, :], psz[:])

                psz3 = ps3.tile([128, 2, 256], F32, tag="ps3", name=f"pz3{b}")
                for h in range(2):
                    nc.tensor.matmul(out=psz3[:, h, :], lhsT=onesr[:],
                                     rhs=b3row, start=True, stop=False)
                    for dc in range(2):
                        nc.tensor.matmul(
                            out=psz3[:, h, :],
                            lhsT=xT[:, dc, b * 256 + h * 128: b * 256 + (h + 1) * 128],
                            rhs=w123[:, dc, 512:768],
                            start=False, stop=(dc == 1))
                act_copy(z3b[:, b, :, :], psz3[:])
                for h in range(2):
                    nc.vector.bn_stats(out=st12[:, b, h, :],
                                       in_=z12b[:, b, h, :])
                    nc.vector.bn_stats(out=st3[:, b, h, :],
                                       in_=z3b[:, b, h, :])
        ps3_cm.__exit__(None, None, None)
        ps12_cm.__exit__(None, None, None)
        trp_cm.__exit__(None, None, None)

        # ---- stats aggregation: per channel mean & E[z^2] ---------------
        def stats_cols_il(lidx, st, arin, col0):
            """st: [128, B_LOC, 2, 6] bn_stats rows of interleaved 512-wide
            inputs; stream lidx(0/1) = cols 3l+1 (mean), 3l+2 (count*var),
            count=256.  arin[:, col0+2h]=mean, +2h+1=E[z^2] (core-local)."""
            for h in range(2):
                msum = small.tile([128, 1], F32, tag=f"ms{lidx}_{h}",
                                  name=f"ms{lidx}{h}")
                nc.vector.tensor_reduce(
                    out=msum[:], in_=st[:, :, h, 3 * lidx + 1:3 * lidx + 2],
                    axis=mybir.AxisListType.XY, op=mybir.AluOpType.add)
                sq = small.tile([128, B_LOC, 1], F32, tag=f"sq{lidx}_{h}",
                                name=f"sq{lidx}{h}")
                nc.vector.tensor_tensor(
                    out=sq[:], in0=st[:, :, h, 3 * lidx + 1:3 * lidx + 2],
                    in1=st[:, :, h, 3 * lidx + 1:3 * lidx + 2],
                    op=mybir.AluOpType.mult)
                sqsum = small.tile([128, 1], F32, tag=f"qs{lidx}_{h}",
                                   name=f"qs{lidx}{h}")
                nc.vector.tensor_reduce(out=sqsum[:], in_=sq[:],
                                        axis=mybir.AxisListType.XY,
                                        op=mybir.AluOpType.add)
                cvsum = small.tile([128, 1], F32, tag=f"cv{lidx}_{h}",
                                   name=f"cv{lidx}{h}")
                nc.vector.tensor_reduce(
                    out=cvsum[:], in_=st[:, :, h, 3 * lidx + 2:3 * lidx + 3],
                    axis=mybir.AxisListType.XY, op=mybir.AluOpType.add)
                nc.vector.tensor_scalar_mul(
                    arin[:, col0 + 2 * h:col0 + 2 * h + 1], msum[:], 1.0 / B_LOC)
                nc.vector.tensor_scalar_mul(cvsum[:], cvsum[:],
                                            1.0 / (256 * B_LOC))
                nc.vector.tensor_scalar_mul(
                    arin[:, col0 + 2 * h + 1:col0 + 2 * h + 2], sqsum[:],
                    1.0 / B_LOC)
                nc.vector.tensor_tensor(
                    out=arin[:, col0 + 2 * h + 1:col0 + 2 * h + 2],
                    in0=arin[:, col0 + 2 * h + 1:col0 + 2 * h + 2],
                    in1=cvsum[:], op=mybir.AluOpType.add)

        def stats_cols(lidx, st, arin, col0, nfree):
            """st: [128, B_LOC, 2, 6] per-batch bn_stats rows over 256-wide
            inputs (even/odd streams, count = nfree/2 each).  Writes per h:
            arin[:, col0+2h] = core mean, +2h+1 = core E[z^2]."""
            ngrp = 2 * B_LOC  # even/odd x batches
            for h in range(2):
                msum = small.tile([128, 1], F32, tag=f"Ms{lidx}_{h}",
                                  name=f"Ms{lidx}{h}")
                nc.vector.tensor_reduce(out=msum[:], in_=st[:, :, h, 1:5:3],
                                        axis=mybir.AxisListType.XY,
                                        op=mybir.AluOpType.add)
                sq = small.tile([128, B_LOC, 2], F32, tag=f"Sq{lidx}_{h}",
                                name=f"Sq{lidx}{h}")
                nc.vector.tensor_tensor(out=sq[:], in0=st[:, :, h, 1:5:3],
                                        in1=st[:, :, h, 1:5:3],
                                        op=mybir.AluOpType.mult)
                sqsum = small.tile([128, 1], F32, tag=f"Qs{lidx}_{h}",
                                   name=f"Qs{lidx}{h}")
                nc.vector.tensor_reduce(out=sqsum[:], in_=sq[:],
                                        axis=mybir.AxisListType.XY,
                                        op=mybir.AluOpType.add)
                cvsum = small.tile([128, 1], F32, tag=f"Cv{lidx}_{h}",
                                   name=f"Cv{lidx}{h}")
                nc.vector.tensor_reduce(out=cvsum[:], in_=st[:, :, h, 2:6:3],
                                        axis=mybir.AxisListType.XY,
                                        op=mybir.AluOpType.add)
                nc.vector.tensor_scalar_mul(
                    arin[:, col0 + 2 * h:col0 + 2 * h + 1], msum[:], 1.0 / ngrp)
                nc.vector.tensor_scalar_mul(
                    cvsum[:], cvsum[:], 1.0 / (ngrp * nfree // 2))
                nc.vector.tensor_scalar_mul(
                    arin[:, col0 + 2 * h + 1:col0 + 2 * h + 2], sqsum[:],
                    1.0 / ngrp)
                nc.vector.tensor_tensor(
                    out=arin[:, col0 + 2 * h + 1:col0 + 2 * h + 2],
                    in0=arin[:, col0 + 2 * h + 1:col0 + 2 * h + 2],
                    in1=cvsum[:], op=mybir.AluOpType.add)

        def emit_allreduce(lidx, arin, width):
            ar_i = dram.tile([128, width], F32, tag=f"ari{lidx}", name=f"ai{lidx}")
            ar_o = dram.tile([128, width], F32, tag=f"aro{lidx}", name=f"ao{lidx}")
            nc.sync.dma_start(out=ar_i[:], in_=arin[:])
            if use_collectives:
                nc.gpsimd.collective_compute(
                    "AllReduce", mybir.AluOpType.add, replica_groups=groups,
                    ins=[ar_i[:].opt()], outs=[ar_o[:].opt()])
            else:
                nc.gpsimd.dma_start(out=ar_o[:], in_=ar_i[:])
            artot = small.tile([128, width], F32, tag=f"artot{lidx}",
                               name=f"at{lidx}")
            nc.sync.dma_start(out=artot[:], in_=ar_o[:])
            return artot

        def bn_finalize(lidx, artot):
            """artot cols per half h: 2h = sum of core means, 2h+1 = sum of
            core E[z^2] (z includes bias).  -> s, c [128, 2] f32."""
            meanz = small.tile([128, 2], F32, tag=f"mz{lidx}", name=f"mz{lidx}")
            nc.vector.tensor_scalar_mul(meanz[:], artot[:, 0:4:2], 1.0 / NCORES)
            ey2 = small.tile([128, 2], F32, tag=f"ey{lidx}", name=f"ey{lidx}")
            nc.vector.tensor_scalar_mul(ey2[:], artot[:, 1:4:2], 1.0 / NCORES)
            varz = small.tile([128, 2], F32, tag=f"vz{lidx}", name=f"vz{lidx}")
            m2 = small.tile([128, 2], F32, tag=f"m2{lidx}", name=f"m2{lidx}")
            nc.vector.tensor_tensor(out=m2[:], in0=meanz[:], in1=meanz[:],
                                    op=mybir.AluOpType.mult)
            nc.vector.tensor_tensor(out=varz[:], in0=ey2[:], in1=m2[:],
                                    op=mybir.AluOpType.subtract)
            nc.vector.tensor_scalar_add(varz[:], varz[:], EPS)
            sd = small.tile([128, 2], F32, tag=f"sd{lidx}", name=f"sd{lidx}")
            nc.scalar.sqrt(out=sd[:], in_=varz[:])
            rstd = small.tile([128, 2], F32, tag=f"rs{lidx}", name=f"rs{lidx}")
            nc.vector.reciprocal(out=rstd[:], in_=sd[:])
            s = small.tile([128, 2], F32, tag=f"s{lidx}", name=f"s{lidx}")
            nc.vector.tensor_tensor(out=s[:], in0=rstd[:], in1=gbt[:, :, 0],
                                    op=mybir.AluOpType.mult)
            c = small.tile([128, 2], F32, tag=f"c{lidx}", name=f"c{lidx}")
            nc.vector.tensor_tensor(out=c[:], in0=meanz[:], in1=s[:],
                                    op=mybir.AluOpType.mult)
            nc.vector.tensor_tensor(out=c[:], in0=gbt[:, :, 1], in1=c[:],
                                    op=mybir.AluOpType.subtract)
            return s, c

        arin123 = small.tile([128, 12], F32, tag="arin123")
        stats_cols_il(0, st12, arin123, 0)
        stats_cols_il(1, st12, arin123, 4)
        stats_cols(2, st3, arin123, 8, 256)
        artot123 = emit_allreduce(123, arin123, 12)
        s_l, c_l = [None] * 4, [None] * 4
        s_l[0], c_l[0] = bn_finalize(0, artot123[:, 0:4])
        s_l[1], c_l[1] = bn_finalize(1, artot123[:, 4:8])
        s_l[2], c_l[2] = bn_finalize(2, artot123[:, 8:12])

        # ---- P3: per-batch apply + transposes + attention + layer4 ------
        st4 = small.tile([128, B_LOC, 2, 6], F32, tag="st12")  # aliases st12 (dead)

        pss_cm = tc.tile_pool(name="pss", bufs=2, space="PSUM")
        pss_p = pss_cm.__enter__()
        psr_cm = tc.tile_pool(name="psr", bufs=2, space="PSUM")
        psr_p = psr_cm.__enter__()
        psy_cm = tc.tile_pool(name="psy", bufs=2, space="PSUM")
        psy_p = psy_cm.__enter__()
        ptr_cm = tc.tile_pool(name="ptr", bufs=2, space="PSUM")
        ptr_p = ptr_cm.__enter__()

        for b in range(B_LOC):
            # -- BN+ReLU applies: x1 (ACT), x2 (GPSIMD), x3 (ACT) --------
            apl = stage.tile([128, 2, 2, 256], BF16, tag="apl", name=f"ap{b}")
            for h in range(2):
                nc.scalar.activation(
                    out=apl[:, 0, h, :], in_=z12b[:, b, h, 0:512:2],
                    func=mybir.ActivationFunctionType.Relu,
                    scale=s_l[0][:, h:h + 1], bias=c_l[0][:, h:h + 1])
            for h in range(2):
                nc.scalar.activation(
                    out=apl[:, 1, h, :], in_=z12b[:, b, h, 1:512:2],
                    func=mybir.ActivationFunctionType.Relu,
                    scale=s_l[1][:, h:h + 1], bias=c_l[1][:, h:h + 1])
            for h in range(2):
                nc.scalar.activation(
                    out=x3a[:, b, h, 0:256], in_=z3b[:, b, h, :],
                    func=mybir.ActivationFunctionType.Relu,
                    scale=s_l[2][:, h:h + 1], bias=c_l[2][:, h:h + 1])
            # -- transpose x1/x2 via PE (bf16 psum) -----------------------
            x1t = stage.tile([128, 2, 256], BF16, tag="x1t", name=f"x1t{b}")
            x2t = stage.tile([128, 2, 256], BF16, tag="x2t", name=f"x2t{b}")
            for l, xlt in ((0, x1t), (1, x2t)):
                pst = ptr_p.tile([128, 2, 2, 128], BF16, tag="ptr",
                                 name=f"ptx{b}_{l}")
                for h in range(2):
                    for dc in range(2):
                        nc.tensor.transpose(
                            out=pst[:, dc, h, :],
                            in_=apl[:, l, h, dc * 128:(dc + 1) * 128],
                            identity=idn[:])
                nc.vector.tensor_copy(
                    out=xlt[:],
                    in_=pst[:].rearrange("p dc h t -> p dc (h t)"))
            # -- S^T = x2^T x1 (per batch) --------------------------------
            pss = pss_p.tile([128, 2, 256], F32, tag="pss", name=f"ss{b}")
            for mc in range(2):
                for ec in range(2):
                    nc.tensor.matmul(
                        out=pss[:, mc, :],
                        lhsT=x2t[:, ec, mc * 128:(mc + 1) * 128],
                        rhs=x1t[:, ec, :],
                        start=(ec == 0), stop=(ec == 1))
            pt = stage.tile([128, 2, 256], BF16, tag="pt", name=f"pt{b}")
            nc.scalar.activation(out=pt[:], in_=pss[:], scale=1.0 / 16.0,
                                 func=mybir.ActivationFunctionType.Exp)
            # -- AV with ones column -> row sums in col 256 ---------------
            rst = stage.tile([128, 2, 257], BF16, tag="rst", name=f"rt{b}")
            invr2 = stage.tile([128, 2], F32, tag="invr", name=f"iv{b}")
            for nc_ in range(2):
                psr = psr_p.tile([128, 257], F32, tag="psr", name=f"pr{b}_{nc_}")
                for mc in range(2):
                    nc.tensor.matmul(
                        out=psr[:, 0:257],
                        lhsT=pt[:, mc, nc_ * 128:(nc_ + 1) * 128],
                        rhs=x3a[:, b, mc, 0:257],
                        start=(mc == 0), stop=(mc == 1))
                nc.vector.reciprocal(out=invr2[:, nc_:nc_ + 1],
                                     in_=psr[:, 256:257])
                nc.vector.tensor_scalar_mul(rst[:, nc_, 0:256], psr[:, 0:256],
                                            invr2[:, nc_:nc_ + 1])
            # -- rT via PE transpose (bf16 psum) --------------------------
            rT = stage.tile([128, 2, 256], BF16, tag="rT", name=f"rr{b}")
            pstr = ptr_p.tile([128, 2, 2, 128], BF16, tag="ptr",
                              name=f"ptr{b}")
            for nc_ in range(2):
                for dc in range(2):
                    nc.tensor.transpose(
                        out=pstr[:, dc, nc_, :],
                        in_=rst[:, nc_, dc * 128:(dc + 1) * 128],
                        identity=idn[:])
            nc.vector.tensor_copy(
                out=rT[:], in_=pstr[:].rearrange("p dc n t -> p dc (n t)"))
            # -- layer 4 (+b4 rank-1), z4 -> x3a storage, stats -----------
            psy = psy_p.tile([128, 2, 256], F32, tag="psy", name=f"py{b}")
            for h in range(2):
                nc.tensor.matmul(out=psy[:, h, :], lhsT=onesr[:], rhs=b4row,
                                 start=True, stop=False)
                for dc in range(2):
                    nc.tensor.matmul(
                        out=psy[:, h, :],
                        lhsT=rT[:, dc, h * 128:(h + 1) * 128],
                        rhs=w4[:, dc, :],
                        start=False, stop=(dc == 1))
            if b % 2 == 0:
                act_copy(x3a[:, b, :, 0:256], psy[:])
            else:
                nc.vector.tensor_copy(out=x3a[:, b, :, 0:256], in_=psy[:])
            for h in range(2):
                nc.vector.bn_stats(out=st4[:, b, h, :],
                                   in_=x3a[:, b, h, 0:256])

        ptr_cm.__exit__(None, None, None)
        psy_cm.__exit__(None, None, None)
        psr_cm.__exit__(None, None, None)
        pss_cm.__exit__(None, None, None)

        # ---- final BN stats AR + apply + store --------------------------
        arin4 = small.tile([128, 4], F32, tag="arin4")
        stats_cols(4, st4, arin4, 0, 256)
        artot4 = emit_allreduce(4, arin4, 4)
        s_l[3], c_l[3] = bn_finalize(3, artot4)
        for b in range(B_LOC):
            ost = stage.tile([128, 2, 256], BF16, tag="ost", bufs=4,
                             name=f"os{b}")
            for h in range(2):
                if b % 3 != 2:
                    nc.scalar.activation(
                        out=ost[:, h, :], in_=x3a[:, b, h, 0:256],
                        func=mybir.ActivationFunctionType.Relu,
                        scale=s_l[3][:, h:h + 1], bias=c_l[3][:, h:h + 1])
                else:
                    nc.vector.tensor_scalar(
                        out=ost[:, h, :], in0=x3a[:, b, h, 0:256],
                        scalar1=s_l[3][:, h:h + 1], scalar2=c_l[3][:, h:h + 1],
                        op0=mybir.AluOpType.mult, op1=mybir.AluOpType.add)
                    nc.vector.tensor_scalar_max(ost[:, h, :], ost[:, h, :],
                                                0.0)
            eng = nc.sync if b % 2 == 0 else nc.scalar
            eng.dma_start(out=out_r[:, b, :, :], in_=ost[:])

    _legalize_waits(nc)
    return nc


_CACHE = {}


def _prep_core_inputs(inputs):
    bf = ml_dtypes.bfloat16
    W = [inputs["W1"], inputs["W2"], inputs["W3"], inputs["W4"]]
    bs = [inputs["b1"], inputs["b2"], inputs["b3"], inputs["b4"]]
    gamma, beta = inputs["gamma"], inputs["beta"]

    w123 = np.zeros((128, 2, 768), dtype=bf)
    for c in range(2):
        w123[:, c, 0:512:2] = W[0][:, c * 128:(c + 1) * 128].T.astype(bf)
        w123[:, c, 1:512:2] = W[1][:, c * 128:(c + 1) * 128].T.astype(bf)
        w123[:, c, 512:768] = W[2][:, c * 128:(c + 1) * 128].T.astype(bf)
    w4 = np.zeros((128, 2, 256), dtype=bf)
    for c in range(2):
        w4[:, c, :] = W[3][:, c * 128:(c + 1) * 128].T.astype(bf)
    brows = np.zeros((1, 1024), dtype=bf)
    brows[0, 0:512:2] = bs[0].astype(bf)
    brows[0, 1:512:2] = bs[1].astype(bf)
    brows[0, 512:768] = bs[2].astype(bf)
    brows[0, 768:1024] = bs[3].astype(bf)
    gb = np.zeros((128, 2, 2), dtype=np.float32)
    for h in range(2):
        gb[:, h, 0] = gamma[h * 128:(h + 1) * 128]
        gb[:, h, 1] = beta[h * 128:(h + 1) * 128]
    return w123, w4, brows, gb


def kernel(**inputs):
    _install_profile_shim()
    from concourse.bass_utils import run_bass_kernel_spmd

    if "nc" not in _CACHE:
        _CACHE["nc"] = build_program()
    nc = _CACHE["nc"]

    x = np.asarray(inputs["x"], dtype=np.float32)
    w123, w4, brows, gb = _prep_core_inputs(
        {k: np.asarray(v) for k, v in inputs.items()})

    in_maps = []
    for i in range(NCORES):
        xs = np.ascontiguousarray(
            x[i * B_LOC:(i + 1) * B_LOC].reshape(T, 256))
        in_maps.append({"x": xs, "w123": w123, "w4": w4, "brows": brows,
                        "gb": gb})

    trace = _CACHE.get("trace", False)
    res = run_bass_kernel_spmd(nc, in_maps, list(range(NCORES)), trace=trace)
    _CACHE["last_result"] = res

    out = np.empty((256, 256, 256), dtype=np.float32)
    for i in range(NCORES):
        out[i * B_LOC:(i + 1) * B_LOC] = np.asarray(
            res.results[i]["out"]).astype(np.float32).reshape(B_LOC, 256, 256)
    return out


# revision 26
# speedup vs baseline: 1.2709x; 1.1348x over previous
"""Fused attention-block kernel for Trainium2, 8-core data-parallel over batch.

Computation (see harness reference): three BN+ReLU linear branches from the
same input, attention (QK^T/16 -> softmax -> AV), then a fourth BN+ReLU
linear.  BatchNorm1d is training-mode per-channel over (batch, feature) with
channel = sequence position, so batch-sharding needs a cross-core stats
all-reduce (sync-BN); weights are replicated.

v2 design notes (vs the v1 baseline at 468us):
 - A tiny dummy AllReduce is issued right after the input-cast DMAs: the
   *first* collective pays a ~90us firmware warmup/skew cost; later ones
   take ~9-13us.  The warmup overlaps the input DMA + z-phase.
 - One merged stats AllReduce for layers 1-3 (instead of two serialized).
 - x is cast fp32->bf16 straight into SBUF (gpsimd SWDGE cast DMA, no DRAM
   round trip) and transposed by the PE during the otherwise-idle prologue
   (also warms the HAM clock gate).
 - All linear-layer biases are pre-added via K=1 rank-1 matmuls (ones row x
   bias row), which makes every BN+ReLU apply a single ACT op with
   per-partition scale/bias, and makes all BN statistics exact/direct.
 - x1/x2/r transposes go through SBUF->SBUF DMA xbar transposes (128x128
   blocks) instead of PE matmuls: frees the tensor engine + PSUM banks so
   the attention pipeline can run ~2 batches deep (pss/psr/psy pools).
 - All attention intermediates are batch-local rotating staging tiles.
 - z4 is written into x3a's storage (x3 of batch b is dead once AV(b) ran).
 - Per-batch work is spread across ACT/DVE/GPSIMD; gpsimd only runs while
   no collective is in flight (device-hang hazard).
 - Output is stored as bf16 (halves store traffic; host casts to fp32).

Hardcoded: B=256, N=256, D=256, 8 cores -> 32 batches (8192 tokens) per core.
"""
import sys
import types

sys.path.insert(0, "/opt/trn_rl_repo")

import numpy as np
import ml_dtypes
from contextlib import ExitStack

import concourse.bass as bass
import concourse.mybir as mybir
import concourse.tile as tile
from concourse.masks import make_identity

BF16 = mybir.dt.bfloat16
F32 = mybir.dt.float32
NCORES = 8
B_LOC = 32          # batches per core
T = B_LOC * 256     # tokens per core
EPS = 1e-5
NG = 2              # stats batch-group size (bn_stats free max = 512)
NCH = B_LOC // 4    # cast/transpose chunks (4 batches each)


def _install_profile_shim():
    """run_bass_kernel_spmd(trace=True) under axon needs antenv.axon_hooks,
    which this image lacks; synthesize it (harmless if tracing unused)."""
    if "antenv.axon_hooks" in sys.modules:
        return
    try:
        import antenv
        mod = types.ModuleType("antenv.axon_hooks")
        mod._hook = None
        mod.set_axon_ntff_profile_hook = lambda h: setattr(mod, "_hook", h)
        mod.get_axon_ntff_profile_hook = lambda: mod._hook
        sys.modules["antenv.axon_hooks"] = mod
        antenv.axon_hooks = mod
        from trn_agent_boot.trn_boot import _ntff_profile_via_ctypes
        hook = _ntff_profile_via_ctypes("/opt/axon/libaxon_pjrt.so")
        if hook is not None:
            mod.set_axon_ntff_profile_hook(hook)
    except Exception:
        pass


def _legalize_waits(nc, max_waits=1):
    """HW instructions carry one sync-wait slot; walrus rejects instructions
    with too many waits.  Hoist extras onto engine-matched NoOps."""
    for f in nc.m.functions:
        for bb in f.blocks:
            insts = bb.instructions
            new_list = []
            for inst in insts:
                si = inst.sync_info
                if si is not None and len(si.on_wait) > max_waits:
                    waits = list(si.on_wait)
                    extra, keep = waits[:-max_waits], waits[-max_waits:]
                    for j, w in enumerate(extra):
                        nop = mybir.InstNoOp(
                            name=f"{inst.name}-waitnop{j}",
                            engine=inst.engine,
                            ins=[], outs=[],
                            sync_info=mybir.SyncInfo(on_wait=[w], on_update=[]),
                        )
                        nc.register_instruction(nop, overwrite=True)
                        new_list.append(nop)
                    inst.sync_info = mybir.SyncInfo(
                        on_wait=keep, on_update=list(si.on_update))
                new_list.append(inst)
            del insts[:]
            for x in new_list:
                insts.append(x)


def build_program(use_collectives=True):
    nc = bass.Bass("TRN2", target_bir_lowering=False, debug=False,
                   num_devices=NCORES)

    def act_copy(out, in_):
        nc.scalar.activation(out=out, in_=in_,
                             func=mybir.ActivationFunctionType.Copy)

    x_d = nc.dram_tensor("x", [T, 256], F32, kind="ExternalInput")
    w123_d = nc.dram_tensor("w123", [128, 2, 768], BF16, kind="ExternalInput")
    w4_d = nc.dram_tensor("w4", [128, 2, 256], BF16, kind="ExternalInput")
    brows_d = nc.dram_tensor("brows", [1, 1024], BF16, kind="ExternalInput")
    gb_d = nc.dram_tensor("gb", [128, 2, 2], F32, kind="ExternalInput")
    out_d = nc.dram_tensor("out", [T, 256], BF16, kind="ExternalOutput")


    groups = [list(range(NCORES))]
    # x rows are (b, h, p); chunks of 4 batches = 8 bh-blocks
    x_r = x_d.ap().rearrange("(c q p) e -> p c q e", c=NCH, q=8, p=128)
    out_r = out_d.ap().rearrange("(b h p) e -> p b h e", b=B_LOC, h=2, p=128)

    with ExitStack() as ctx:
        tc = ctx.enter_context(tile.TileContext(nc))
        big = ctx.enter_context(tc.tile_pool(name="big", bufs=1))
        small = ctx.enter_context(tc.tile_pool(name="small", bufs=1))
        stage = ctx.enter_context(tc.tile_pool(name="stage", bufs=3))
        dram = ctx.enter_context(tc.tile_pool(name="dram", bufs=1, space="DRAM"))

        # ---- persistent big tiles --------------------------------------
        xT = big.tile([128, 2, T], BF16, tag="xT")              # x d-major
        z12b = big.tile([128, B_LOC, 2, 2, 256], BF16, tag="z12")  # (b,h,l,e)
        z3b = big.tile([128, B_LOC, 2, 256], BF16, tag="z3")
        x3a = big.tile([128, B_LOC, 2, 257], BF16, tag="x3a")   # +ones col 256

        # ---- constants --------------------------------------------------
        w123 = small.tile([128, 2, 768], BF16, tag="w123")
        w4 = small.tile([128, 2, 256], BF16, tag="w4")
        brows = small.tile([1, 1024], BF16, tag="brows")
        gbt = small.tile([128, 2, 2], F32, tag="gbt")
        idn = small.tile([128, 128], BF16, tag="idn")
        onesr = small.tile([1, 128], BF16, tag="onesr")
        nc.sync.dma_start(out=w123[:], in_=w123_d.ap())
        nc.sync.dma_start(out=w4[:], in_=w4_d.ap())
        nc.sync.dma_start(out=brows[:], in_=brows_d.ap())
        nc.sync.dma_start(out=gbt[:], in_=gb_d.ap())
        make_identity(nc, idn[:])
        nc.vector.memset(onesr[:], 1.0)
        nc.vector.memset(x3a[:, :, :, 256:257], 1.0)
        b12row = brows[:, 0:512]      # [b1 | b2]
        b3row = brows[:, 512:768]
        b4row = brows[:, 768:1024]

        # ---- x cast DMAs (gpsimd SWDGE), then dummy warmup AllReduce ----
        # Casts are emitted strictly before the collective trigger: pool-
        # engine work concurrent with an in-flight collective hangs.
        xstg = []
        for c in range(NCH):
            xs = stage.tile([128, 8, 256], BF16, tag="xstg", bufs=2,
                            name=f"xs{c}")
            nc.gpsimd.dma_start(out=xs[:], in_=x_r[:, c, :, :])
            xstg.append(xs)
        ar_w_i = dram.tile([128, 2], F32, tag="arwi")
        ar_w_o = dram.tile([128, 2], F32, tag="arwo")
        if use_collectives:
            nc.gpsimd.collective_compute(
                "AllReduce", mybir.AluOpType.add, replica_groups=groups,
                ins=[ar_w_i[:].opt()], outs=[ar_w_o[:].opt()])

        # ---- P0+P1 interleaved per chunk: transpose x, z12+z3 matmuls ---
        st1 = small.tile([128, B_LOC, 2, 6], F32, tag="st1")  # per-batch rows
        st2 = small.tile([128, B_LOC, 2, 6], F32, tag="st2")
        st3 = small.tile([128, B_LOC, 2, 6], F32, tag="st3")
        trp_cm = tc.tile_pool(name="trp", bufs=2, space="PSUM")
        trp = trp_cm.__enter__()
        ps12_cm = tc.tile_pool(name="ps12", bufs=4, space="PSUM")
        ps12 = ps12_cm.__enter__()
        ps3_cm = tc.tile_pool(name="ps3", bufs=2, space="PSUM")
        ps3 = ps3_cm.__enter__()
        for c in range(NCH):
            for q in range(8):
                bh = c * 8 + q
                pt0 = trp.tile([128, 2, 128], BF16, tag="trp", name=f"tp{bh}")
                for dc in range(2):
                    nc.tensor.transpose(
                        out=pt0[:, dc, :],
                        in_=xstg[c][:, q, dc * 128:(dc + 1) * 128],
                        identity=idn[:])
                if bh % 2 == 0:
                    nc.vector.tensor_copy(out=xT[:, :, bh * 128:(bh + 1) * 128],
                                          in_=pt0[:])
                else:
                    act_copy(xT[:, :, bh * 128:(bh + 1) * 128], pt0[:])
            for b in range(c * 4, (c + 1) * 4):
                # shared-lhsT ordering per h: ones-bias MMs for z12 and z3,
                # then per dc one z12 and one z3 MM off the same xT block.
                for h in range(2):
                    psz = ps12.tile([128, 512], F32, tag="ps12",
                                    name=f"pz{b}_{h}")
                    psz3 = ps3.tile([128, 256], F32, tag="ps3",
                                    name=f"pz3{b}_{h}")
                    nc.tensor.matmul(out=psz[:], lhsT=onesr[:],
                                     rhs=b12row, start=True, stop=False)
                    nc.tensor.matmul(out=psz3[:], lhsT=onesr[:],
                                     rhs=b3row, start=True, stop=False)
                    for dc in range(2):
                        xblk = xT[:, dc,
                                  b * 256 + h * 128: b * 256 + (h + 1) * 128]
                        nc.tensor.matmul(out=psz[:], lhsT=xblk,
                                         rhs=w123[:, dc, 0:512],
                                         start=False, stop=(dc == 1))
                        nc.tensor.matmul(out=psz3[:], lhsT=xblk,
                                         rhs=w123[:, dc, 512:768],
                                         start=False, stop=(dc == 1))
                    act_copy(z12b[:, b, h, :, :], psz[:])
                    act_copy(z3b[:, b, h, :], psz3[:])
                    nc.vector.bn_stats(out=st1[:, b, h, :],
                                       in_=z12b[:, b, h, 0, :])
                    nc.vector.bn_stats(out=st2[:, b, h, :],
                                       in_=z12b[:, b, h, 1, :])
                    nc.vector.bn_stats(out=st3[:, b, h, :],
                                       in_=z3b[:, b, h, :])
        ps3_cm.__exit__(None, None, None)
        ps12_cm.__exit__(None, None, None)
        trp_cm.__exit__(None, None, None)

        # ---- stats aggregation: per channel mean & E[z^2] ---------------
        def stats_cols_il(lidx, st, arin, col0):
            """st: [128, B_LOC, 2, 6] bn_stats rows of interleaved 512-wide
            inputs; stream lidx(0/1) = cols 3l+1 (mean), 3l+2 (count*var),
            count=256.  arin[:, col0+2h]=mean, +2h+1=E[z^2] (core-local)."""
            for h in range(2):
                msum = small.tile([128, 1], F32, tag=f"ms{lidx}_{h}",
                                  name=f"ms{lidx}{h}")
                nc.vector.tensor_reduce(
                    out=msum[:], in_=st[:, :, h, 3 * lidx + 1:3 * lidx + 2],
                    axis=mybir.AxisListType.XY, op=mybir.AluOpType.add)
                sq = small.tile([128, B_LOC, 1], F32, tag=f"sq{lidx}_{h}",
                                name=f"sq{lidx}{h}")
                nc.vector.tensor_tensor(
                    out=sq[:], in0=st[:, :, h, 3 * lidx + 1:3 * lidx + 2],
                    in1=st[:, :, h, 3 * lidx + 1:3 * lidx + 2],
                    op=mybir.AluOpType.mult)
                sqsum = small.tile([128, 1], F32, tag=f"qs{lidx}_{h}",
                                   name=f"qs{lidx}{h}")
                nc.vector.tensor_reduce(out=sqsum[:], in_=sq[:],
                                        axis=mybir.AxisListType.XY,
                                        op=mybir.AluOpType.add)
                cvsum = small.tile([128, 1], F32, tag=f"cv{lidx}_{h}",
                                   name=f"cv{lidx}{h}")
                nc.vector.tensor_reduce(
                    out=cvsum[:], in_=st[:, :, h, 3 * lidx + 2:3 * lidx + 3],
                    axis=mybir.AxisListType.XY, op=mybir.AluOpType.add)
                nc.vector.tensor_scalar_mul(
                    arin[:, col0 + 2 * h:col0 + 2 * h + 1], msum[:], 1.0 / B_LOC)
                nc.vector.tensor_scalar_mul(cvsum[:], cvsum[:],
                                            1.0 / (256 * B_LOC))
                nc.vector.tensor_scalar_mul(
                    arin[:, col0 + 2 * h + 1:col0 + 2 * h + 2], sqsum[:],
                    1.0 / B_LOC)
                nc.vector.tensor_tensor(
                    out=arin[:, col0 + 2 * h + 1:col0 + 2 * h + 2],
                    in0=arin[:, col0 + 2 * h + 1:col0 + 2 * h + 2],
                    in1=cvsum[:], op=mybir.AluOpType.add)

        def stats_cols(lidx, st, arin, col0, nfree):
            """st: [128, B_LOC, 2, 6] per-batch bn_stats rows over 256-wide
            inputs (even/odd streams, count = nfree/2 each).  Writes per h:
            arin[:, col0+2h] = core mean, +2h+1 = core E[z^2]."""
            ngrp = 2 * B_LOC  # even/odd x batches
            for h in range(2):
                msum = small.tile([128, 1], F32, tag=f"Ms{lidx}_{h}",
                                  name=f"Ms{lidx}{h}")
                nc.vector.tensor_reduce(out=msum[:], in_=st[:, :, h, 1:5:3],
                                        axis=mybir.AxisListType.XY,
                                        op=mybir.AluOpType.add)
                sq = small.tile([128, B_LOC, 2], F32, tag=f"Sq{lidx}_{h}",
                                name=f"Sq{lidx}{h}")
                nc.vector.tensor_tensor(out=sq[:], in0=st[:, :, h, 1:5:3],
                                        in1=st[:, :, h, 1:5:3],
                                        op=mybir.AluOpType.mult)
                sqsum = small.tile([128, 1], F32, tag=f"Qs{lidx}_{h}",
                                   name=f"Qs{lidx}{h}")
                nc.vector.tensor_reduce(out=sqsum[:], in_=sq[:],
                                        axis=mybir.AxisListType.XY,
                                        op=mybir.AluOpType.add)
                cvsum = small.tile([128, 1], F32, tag=f"Cv{lidx}_{h}",
                                   name=f"Cv{lidx}{h}")
                nc.vector.tensor_reduce(out=cvsum[:], in_=st[:, :, h, 2:6:3],
                                        axis=mybir.AxisListType.XY,
                                        op=mybir.AluOpType.add)
                nc.vector.tensor_scalar_mul(
                    arin[:, col0 + 2 * h:col0 + 2 * h + 1], msum[:], 1.0 / ngrp)
                nc.vector.tensor_scalar_mul(
                    cvsum[:], cvsum[:], 1.0 / (ngrp * nfree // 2))
                nc.vector.tensor_scalar_mul(
                    arin[:, col0 + 2 * h + 1:col0 + 2 * h + 2], sqsum[:],
                    1.0 / ngrp)
                nc.vector.tensor_tensor(
                    out=arin[:, col0 + 2 * h + 1:col0 + 2 * h + 2],
                    in0=arin[:, col0 + 2 * h + 1:col0 + 2 * h + 2],
                    in1=cvsum[:], op=mybir.AluOpType.add)

        def emit_allreduce(lidx, arin, width):
            ar_i = dram.tile([128, width], F32, tag=f"ari{lidx}", name=f"ai{lidx}")
            ar_o = dram.tile([128, width], F32, tag=f"aro{lidx}", name=f"ao{lidx}")
            nc.sync.dma_start(out=ar_i[:], in_=arin[:])
            if use_collectives:
                nc.gpsimd.collective_compute(
                    "AllReduce", mybir.AluOpType.add, replica_groups=groups,
                    ins=[ar_i[:].opt()], outs=[ar_o[:].opt()])
            else:
                nc.gpsimd.dma_start(out=ar_o[:], in_=ar_i[:])
            artot = small.tile([128, width], F32, tag=f"artot{lidx}",
                               name=f"at{lidx}")
            nc.sync.dma_start(out=artot[:], in_=ar_o[:])
            return artot

        def bn_finalize(lidx, artot):
            """artot cols per half h: 2h = sum of core means, 2h+1 = sum of
            core E[z^2] (z includes bias).  -> s, c [128, 2] f32."""
            meanz = small.tile([128, 2], F32, tag=f"mz{lidx}", name=f"mz{lidx}")
            nc.vector.tensor_scalar_mul(meanz[:], artot[:, 0:4:2], 1.0 / NCORES)
            ey2 = small.tile([128, 2], F32, tag=f"ey{lidx}", name=f"ey{lidx}")
            nc.vector.tensor_scalar_mul(ey2[:], artot[:, 1:4:2], 1.0 / NCORES)
            varz = small.tile([128, 2], F32, tag=f"vz{lidx}", name=f"vz{lidx}")
            m2 = small.tile([128, 2], F32, tag=f"m2{lidx}", name=f"m2{lidx}")
            nc.vector.tensor_tensor(out=m2[:], in0=meanz[:], in1=meanz[:],
                                    op=mybir.AluOpType.mult)
            nc.vector.tensor_tensor(out=varz[:], in0=ey2[:], in1=m2[:],
                                    op=mybir.AluOpType.subtract)
            nc.vector.tensor_scalar_add(varz[:], varz[:], EPS)
            sd = small.tile([128, 2], F32, tag=f"sd{lidx}", name=f"sd{lidx}")
            nc.scalar.sqrt(out=sd[:], in_=varz[:])
            rstd = small.tile([128, 2], F32, tag=f"rs{lidx}", name=f"rs{lidx}")
            nc.vector.reciprocal(out=rstd[:], in_=sd[:])
            s = small.tile([128, 2], F32, tag=f"s{lidx}", name=f"s{lidx}")
            nc.vector.tensor_tensor(out=s[:], in0=rstd[:], in1=gbt[:, :, 0],
                                    op=mybir.AluOpType.mult)
            c = small.tile([128, 2], F32, tag=f"c{lidx}", name=f"c{lidx}")
            nc.vector.tensor_tensor(out=c[:], in0=meanz[:], in1=s[:],
                                    op=mybir.AluOpType.mult)
            nc.vector.tensor_tensor(out=c[:], in0=gbt[:, :, 1], in1=c[:],
                                    op=mybir.AluOpType.subtract)
            return s, c

        arin123 = small.tile([128, 12], F32, tag="arin123")
        stats_cols(0, st1, arin123, 0, 256)
        stats_cols(1, st2, arin123, 4, 256)
        stats_cols(2, st3, arin123, 8, 256)
        artot123 = emit_allreduce(123, arin123, 12)
        s_l, c_l = [None] * 4, [None] * 4
        s_l[0], c_l[0] = bn_finalize(0, artot123[:, 0:4])
        s_l[1], c_l[1] = bn_finalize(1, artot123[:, 4:8])
        s_l[2], c_l[2] = bn_finalize(2, artot123[:, 8:12])

        # ---- PE keep-warm filler across the AR123 bubble ----------------
        # ~110 junk N=512 matmuls (~30us warm); WAW-chained on one bank.
        # Without this the HAM clock-gate re-throttles during the AR and
        # was observed stuck at K=4/8 (1.2 GHz) for the whole attention
        # phase.
        fil_cm = tc.tile_pool(name="fil", bufs=1, space="PSUM")
        fil_p = fil_cm.__enter__()
        fil = fil_p.tile([128, 512], F32, tag="fil")
        for i in range(110):
            nc.tensor.matmul(out=fil[:], lhsT=xT[:, 0, 0:128],
                             rhs=w123[:, 0, 0:512], start=True, stop=True)
        fil_cm.__exit__(None, None, None)

        # ---- P3: per-batch apply + transposes + attention + layer4 ------
        st4 = small.tile([128, B_LOC, 2, 6], F32, tag="st1")  # aliases st1 (dead)

        psA_cm = tc.tile_pool(name="psA", bufs=4, space="PSUM")
        psA_p = psA_cm.__enter__()
        psr_cm = tc.tile_pool(name="psr", bufs=2, space="PSUM")
        psr_p = psr_cm.__enter__()
        psy_cm = tc.tile_pool(name="psy", bufs=2, space="PSUM")
        psy_p = psy_cm.__enter__()

        for b in range(B_LOC):
            # -- BN+ReLU applies: x1 (ACT), x2 (GPSIMD), x3 (ACT) --------
            apl = stage.tile([128, 2, 2, 256], BF16, tag="apl", name=f"ap{b}")
            for h in range(2):
                nc.scalar.activation(
                    out=apl[:, 0, h, :], in_=z12b[:, b, h, 0, :],
                    func=mybir.ActivationFunctionType.Relu,
                    scale=s_l[0][:, h:h + 1], bias=c_l[0][:, h:h + 1])
            for h in range(2):
                nc.scalar.activation(
                    out=apl[:, 1, h, :], in_=z12b[:, b, h, 1, :],
                    func=mybir.ActivationFunctionType.Relu,
                    scale=s_l[1][:, h:h + 1], bias=c_l[1][:, h:h + 1])
            for h in range(2):
                nc.scalar.activation(
                    out=x3a[:, b, h, 0:256], in_=z3b[:, b, h, :],
                    func=mybir.ActivationFunctionType.Relu,
                    scale=s_l[2][:, h:h + 1], bias=c_l[2][:, h:h + 1])
            # -- transpose x1/x2 via PE (bf16 psum) -----------------------
            x1t = stage.tile([128, 2, 256], BF16, tag="x1t", name=f"x1t{b}")
            x2t = stage.tile([128, 2, 256], BF16, tag="x2t", name=f"x2t{b}")
            pst = psA_p.tile([128, 2, 2, 2, 128], BF16, tag="psA",
                             name=f"ptx{b}")
            for l, xlt in ((0, x1t), (1, x2t)):
                for h in range(2):
                    for dc in range(2):
                        nc.tensor.transpose(
                            out=pst[:, l, dc, h, :],
                            in_=apl[:, l, h, dc * 128:(dc + 1) * 128],
                            identity=idn[:])
                nc.vector.tensor_copy(
                    out=xlt[:],
                    in_=pst[:, l].rearrange("p dc h t -> p dc (h t)"))
            # -- S^T = x2^T x1 (per batch) --------------------------------
            pss = psA_p.tile([128, 2, 256], F32, tag="psA", name=f"ss{b}")
            for mc in range(2):
                for ec in range(2):
                    nc.tensor.matmul(
                        out=pss[:, mc, :],
                        lhsT=x2t[:, ec, mc * 128:(mc + 1) * 128],
                        rhs=x1t[:, ec, :],
                        start=(ec == 0), stop=(ec == 1))
            pt = stage.tile([128, 2, 256], BF16, tag="pt", name=f"pt{b}")
            nc.scalar.activation(out=pt[:], in_=pss[:], scale=1.0 / 16.0,
                                 func=mybir.ActivationFunctionType.Exp)
            # -- AV with ones column -> row sums in col 256 ---------------
            rst = stage.tile([128, 2, 257], BF16, tag="rst", name=f"rt{b}")
            invr2 = stage.tile([128, 2], F32, tag="invr", name=f"iv{b}")
            for nc_ in range(2):
                psr = psr_p.tile([128, 257], F32, tag="psr", name=f"pr{b}_{nc_}")
                for mc in range(2):
                    nc.tensor.matmul(
                        out=psr[:, 0:257],
                        lhsT=pt[:, mc, nc_ * 128:(nc_ + 1) * 128],
                        rhs=x3a[:, b, mc, 0:257],
                        start=(mc == 0), stop=(mc == 1))
                nc.vector.reciprocal(out=invr2[:, nc_:nc_ + 1],
                                     in_=psr[:, 256:257])
                if nc_ == 0:
                    nc.vector.tensor_scalar_mul(rst[:, nc_, 0:256],
                                                psr[:, 0:256],
                                                invr2[:, nc_:nc_ + 1])
                else:
                    nc.scalar.activation(
                        out=rst[:, nc_, 0:256], in_=psr[:, 0:256],
                        func=mybir.ActivationFunctionType.Copy,
                        scale=invr2[:, nc_:nc_ + 1])
            # -- rT via PE transpose (bf16 psum) --------------------------
            rT = stage.tile([128, 2, 256], BF16, tag="rT", name=f"rr{b}")
            pstr = psA_p.tile([128, 2, 2, 128], BF16, tag="psA",
                              name=f"ptr{b}")
            for nc_ in range(2):
                for dc in range(2):
                    nc.tensor.transpose(
                        out=pstr[:, dc, nc_, :],
                        in_=rst[:, nc_, dc * 128:(dc + 1) * 128],
                        identity=idn[:])
            nc.vector.tensor_copy(
                out=rT[:], in_=pstr[:].rearrange("p dc n t -> p dc (n t)"))
            # -- layer 4 (+b4 rank-1), z4 -> x3a storage, stats -----------
            psy = psy_p.tile([128, 2, 256], F32, tag="psy", name=f"py{b}")
            for h in range(2):
                nc.tensor.matmul(out=psy[:, h, :], lhsT=onesr[:], rhs=b4row,
                                 start=True, stop=False)
                for dc in range(2):
                    nc.tensor.matmul(
                        out=psy[:, h, :],
                        lhsT=rT[:, dc, h * 128:(h + 1) * 128],
                        rhs=w4[:, dc, :],
                        start=False, stop=(dc == 1))
            if b % 2 == 0:
                act_copy(x3a[:, b, :, 0:256], psy[:])
            else:
                nc.vector.tensor_copy(out=x3a[:, b, :, 0:256], in_=psy[:])
            for h in range(2):
                nc.vector.bn_stats(out=st4[:, b, h, :],
                                   in_=x3a[:, b, h, 0:256])

        psy_cm.__exit__(None, None, None)
        psr_cm.__exit__(None, None, None)
        psA_cm.__exit__(None, None, None)

        # ---- final BN stats AR + apply + store --------------------------
        arin4 = small.tile([128, 4], F32, tag="arin4")
        stats_cols(4, st4, arin4, 0, 256)
        artot4 = emit_allreduce(4, arin4, 4)
        s_l[3], c_l[3] = bn_finalize(3, artot4)
        for b in range(B_LOC):
            ost = stage.tile([128, 2, 256], BF16, tag="ost", bufs=3,
                             name=f"os{b}")
            for h in range(2):
                if b % 3 != 2:
                    nc.scalar.activation(
                        out=ost[:, h, :], in_=x3a[:, b, h, 0:256],
                        func=mybir.ActivationFunctionType.Relu,
                        scale=s_l[3][:, h:h + 1], bias=c_l[3][:, h:h + 1])
                else:
                    nc.vector.tensor_scalar(
                        out=ost[:, h, :], in0=x3a[:, b, h, 0:256],
                        scalar1=s_l[3][:, h:h + 1], scalar2=c_l[3][:, h:h + 1],
                        op0=mybir.AluOpType.mult, op1=mybir.AluOpType.add)
                    nc.vector.tensor_scalar_max(ost[:, h, :], ost[:, h, :],
                                                0.0)
            eng = nc.sync if b % 2 == 0 else nc.scalar
            eng.dma_start(out=out_r[:, b, :, :], in_=ost[:])

    _legalize_waits(nc)
    return nc


_CACHE = {}


def _prep_core_inputs(inputs):
    bf = ml_dtypes.bfloat16
    W = [inputs["W1"], inputs["W2"], inputs["W3"], inputs["W4"]]
    bs = [inputs["b1"], inputs["b2"], inputs["b3"], inputs["b4"]]
    gamma, beta = inputs["gamma"], inputs["beta"]

    w123 = np.zeros((128, 2, 768), dtype=bf)
    for c in range(2):
        w123[:, c, 0:256] = W[0][:, c * 128:(c + 1) * 128].T.astype(bf)
        w123[:, c, 256:512] = W[1][:, c * 128:(c + 1) * 128].T.astype(bf)
        w123[:, c, 512:768] = W[2][:, c * 128:(c + 1) * 128].T.astype(bf)
    w4 = np.zeros((128, 2, 256), dtype=bf)
    for c in range(2):
        w4[:, c, :] = W[3][:, c * 128:(c + 1) * 128].T.astype(bf)
    brows = np.zeros((1, 1024), dtype=bf)
    brows[0, 0:256] = bs[0].astype(bf)
    brows[0, 256:512] = bs[1].astype(bf)
    brows[0, 512:768] = bs[2].astype(bf)
    brows[0, 768:1024] = bs[3].astype(bf)
    gb = np.zeros((128, 2, 2), dtype=np.float32)
    for h in range(2):
        gb[:, h, 0] = gamma[h * 128:(h + 1) * 128]
        gb[:, h, 1] = beta[h * 128:(h + 1) * 128]
    return w123, w4, brows, gb


def kernel(**inputs):
    _install_profile_shim()
    from concourse.bass_utils import run_bass_kernel_spmd

    if "nc" not in _CACHE:
        _CACHE["nc"] = build_program()
    nc = _CACHE["nc"]

    x = np.asarray(inputs["x"], dtype=np.float32)
    w123, w4, brows, gb = _prep_core_inputs(
        {k: np.asarray(v) for k, v in inputs.items()})

    in_maps = []
    for i in range(NCORES):
        xs = np.ascontiguousarray(
            x[i * B_LOC:(i + 1) * B_LOC].reshape(T, 256))
        in_maps.append({"x": xs, "w123": w123, "w4": w4, "brows": brows,
                        "gb": gb})

    trace = _CACHE.get("trace", False)
    res = run_bass_kernel_spmd(nc, in_maps, list(range(NCORES)), trace=trace)
    _CACHE["last_result"] = res

    out = np.empty((256, 256, 256), dtype=np.float32)
    for i in range(NCORES):
        out[i * B_LOC:(i + 1) * B_LOC] = np.asarray(
            res.results[i]["out"]).astype(np.float32).reshape(B_LOC, 256, 256)
    return out
